# revision 4
# baseline (speedup 1.0000x reference)
"""ChebConv (K=4) Trainium2 kernel — sparse gather formulation.

Math (exactly matches the reference, which applies the spmm to `x` — not the
recurrence state — in every Chebyshev iteration):

    deg   = segment_sum(edge_weight, row)
    dinv  = deg^-1/2 (0 where deg <= 0)
    lap_e = -2*dinv[row_e]*w_e*dinv[col_e]        (per edge, no self loops)
    Lx    = scatter-add of lap_e * x[col_e] into row_e   (per batch)
    out   = x @ (W0 - W2 + 2*fill*Bm) + Lx @ Bm + bias,  Bm = W1 + 2*W2 + W3

Device strategy: shard destination rows over 8 cores (1280 rows = 10 tiles of
128 each).  Per dest tile, dma_gather (SWDGE) pulls the tile's ~2e3 source
rows from HBM (1 KiB each: 4 batches x 128 feats bf16) into SBUF slot-major;
the DVE builds the 128x(KC*128) coefficient matrix C^T[p,k,d] =
val[slot]*(iota[d]==dest[slot]) from two broadcast tensor_tensor ops; the PE
contracts C^T @ Xg into PSUM (KC accumulating 128x128x512 matmuls).  This
replaces the dense 1280x10240x512 matmul of the previous version (~600x more
MACs than the edges require).  Phase 2 (feature transform) is unchanged from
the dense version but runs in bf16.
"""

import numpy as np
import ml_dtypes

B = 4
N_NODES = 10000
F = 128
SELF_LOOP_FILL = -0.05
NCORES = 8
NPAD = 10240                 # 80 dest tiles of 128
NTILES = NPAD // 128         # 80
MROWS = NPAD // NCORES       # 1280 dest rows per core
MT = MROWS // 128            # 10 dest tiles per core
BF = B * F                   # 512 moving columns (4 batches x 128 feats)

_state = {}


def _build_nc(KC):
    """Build the SPMD program; KC = gather chunks (of 128 slots) per tile."""
    from contextlib import ExitStack

    import concourse.bass as bass
    import concourse.bacc as bacc
    import concourse.tile as tile
    from concourse import mybir
    from concourse.library_config import mlp
    from concourse.alu_op_type import AluOpType

    dt = mybir.dt
    NIDX = KC * 128
    nc = bacc.Bacc(
        "TRN2", target_bir_lowering=False, debug=False, num_devices=NCORES
    )

    xrows = nc.declare_dram_parameter("xrows", [N_NODES, BF], dt.bfloat16, isOutput=False)
    idxt = nc.declare_dram_parameter("idxt", [128, MT * 8 * KC], dt.int16, isOutput=False)
    destt = nc.declare_dram_parameter("destt", [128, MT, KC], dt.bfloat16, isOutput=False)
    valt = nc.declare_dram_parameter("valt", [128, MT, KC], dt.bfloat16, isOutput=False)
    xt = nc.declare_dram_parameter("xt", [128, B, MROWS], dt.bfloat16, isOutput=False)
    wa = nc.declare_dram_parameter("wa", [128, 128], dt.bfloat16, isOutput=False)
    wb = nc.declare_dram_parameter("wb", [128, 128], dt.bfloat16, isOutput=False)
    biasv = nc.declare_dram_parameter("biasv", [128, 1], dt.float32, isOutput=False)
    ident = nc.declare_dram_parameter("ident", [128, 128], dt.bfloat16, isOutput=False)
    iotab = nc.declare_dram_parameter("iotab", [128, 128], dt.bfloat16, isOutput=False)
    out_t = nc.declare_dram_parameter("out_t", [B, 128, MROWS], dt.float32, isOutput=True)

    with ExitStack() as ctx:
        tc = ctx.enter_context(tile.TileContext(nc))
        const = ctx.enter_context(tc.tile_pool(name="const", bufs=1))
        xgpool = ctx.enter_context(tc.tile_pool(name="xg", bufs=3))
        ctpool = ctx.enter_context(tc.tile_pool(name="ct", bufs=2))
        lxpool = ctx.enter_context(tc.tile_pool(name="lx", bufs=2))
        lxtpool = ctx.enter_context(tc.tile_pool(name="lxt", bufs=1))
        outpool = ctx.enter_context(tc.tile_pool(name="outstg", bufs=3))
        psum = ctx.enter_context(
            tc.tile_pool(name="psum", bufs=8, space=bass.MemorySpace.PSUM)
        )

        # constants on the scalar HWDGE queue; ident first (PE warmup needs it)
        id_sb = const.tile([128, 128], dt.bfloat16, tag="ident")
        nc.scalar.dma_start(id_sb[:], ident[:])
        iota_sb = const.tile([128, 128], dt.bfloat16, tag="iota")
        nc.scalar.dma_start(iota_sb[:], iotab[:])
        # gather index/coeff tables ride the sync queue (first consumer: pool)
        idx_sb = const.tile([128, MT * 8 * KC], dt.int16, tag="idx")
        nc.sync.dma_start(idx_sb[:], idxt[:])
        dest_sb = const.tile([128, MT, KC], dt.bfloat16, tag="dest")
        nc.sync.dma_start(dest_sb[:], destt[:])
        val_sb = const.tile([128, MT, KC], dt.bfloat16, tag="val")
        nc.sync.dma_start(val_sb[:], valt[:])
        wa_sb = const.tile([128, 128], dt.bfloat16, tag="wa")
        nc.scalar.dma_start(wa_sb[:], wa[:])
        wb_sb = const.tile([128, 128], dt.bfloat16, tag="wb")
        nc.scalar.dma_start(wb_sb[:], wb[:])
        bias_sb = const.tile([128, 1], dt.float32, tag="bias")
        nc.scalar.dma_start(bias_sb[:], biasv[:])
        xt_sb = const.tile([128, B, MROWS], dt.bfloat16, tag="xt")

        lxT_sb = lxtpool.tile([128, B, MROWS], dt.bfloat16)

        # GPSIMD: load the ucode library that carries InstDMAGatherAnt
        nc.gpsimd.load_library(mlp)

        # PE warmup: dummy matmuls open the HAM clock gate / p-state ramp
        pw = psum.tile([128, 128], dt.float32, tag="ps", name="ps_warm")
        for i in range(36):
            nc.tensor.matmul(
                pw[:], id_sb[:], id_sb[:], start=(i == 0), stop=(i == 35)
            )

        # Phase 1: per dest tile, gather source rows + build C^T + matmul
        for t in range(MT):
            xg = xgpool.tile([128, KC, BF], dt.bfloat16, tag="xg")
            # SWDGE ring holds ~1k descriptors: split the tile's gather into
            # sub-gathers of <= 8 chunks (1024 descriptors each)
            for s in range(0, KC, 8):
                nch = min(8, KC - s)
                c0 = t * 8 * KC + s * 8
                nc.gpsimd.dma_gather(
                    xg[:, s : s + nch, :], xrows[:],
                    idx_sb[:, c0 : c0 + 8 * nch],
                    nch * 128, nch * 128, BF,
                )
            ct = ctpool.tile([128, KC, 128], dt.bfloat16, tag="ct")
            iota_bc = iota_sb[:].unsqueeze(1).broadcast_to([128, KC, 128])
            dest_bc = dest_sb[:, t, :].unsqueeze(2).broadcast_to([128, KC, 128])
            val_bc = val_sb[:, t, :].unsqueeze(2).broadcast_to([128, KC, 128])
            nc.vector.tensor_tensor(ct[:], iota_bc, dest_bc, op=AluOpType.is_equal)
            nc.vector.tensor_tensor(ct[:], ct[:], val_bc, op=AluOpType.mult)
            ps = psum.tile([128, BF], dt.float32, tag="ps", name=f"ps1_{t}")
            for k in range(KC):
                nc.tensor.matmul(
                    ps[:], ct[:, k, :], xg[:, k, :],
                    start=(k == 0), stop=(k == KC - 1),
                )
            # Lx tile (node-major) -> bf16 staging on the ACT engine
            lx = lxpool.tile([128, BF], dt.bfloat16, tag="lx")
            nc.scalar.activation(
                lx[:], ps[:], mybir.ActivationFunctionType.Identity
            )
            # transpose to feature-major for phase 2
            for b in range(B):
                pt = psum.tile([128, 128], dt.bfloat16, tag="ps", name=f"pt_{t}_{b}")
                nc.tensor.transpose(
                    pt[:], lx[:, b * 128 : (b + 1) * 128], id_sb[:]
                )
                nc.vector.tensor_copy(lxT_sb[:, b, t * 128 : (t + 1) * 128], pt[:])

        # xt is only needed by phase 2 — load behind the gather stream
        nc.scalar.dma_start(xt_sb[:], xt[:])

        # Phase 2: out_T = A'^T x^T + Bm^T Lx^T + bias
        starts = list(range(0, MROWS, 512))
        for b in range(B):
            for st in starts:
                csz = min(512, MROWS - st)
                ps2 = psum.tile([128, 512], dt.float32, tag="ps", name=f"ps2_{b}_{st}")
                nc.tensor.matmul(
                    ps2[:, :csz], wa_sb[:], xt_sb[:, b, st : st + csz],
                    start=True, stop=False,
                )
                nc.tensor.matmul(
                    ps2[:, :csz], wb_sb[:], lxT_sb[:, b, st : st + csz],
                    start=False, stop=True,
                )
                ot = outpool.tile([128, 512], dt.float32, tag="ot")
                nc.scalar.activation(
                    ot[:, :csz], ps2[:, :csz],
                    mybir.ActivationFunctionType.Identity,
                    bias=bias_sb[:],
                )
                nc.scalar.dma_start(out_t[b, :, st : st + csz], ot[:, :csz])

    return nc


def _get_nc(KC):
    key = ("nc", KC)
    if key not in _state:
        nc = _build_nc(KC)
        nc.compile()
        _state[key] = nc
    return _state[key]


def _prep_inputs(x, edge_index, edge_weight, weight, bias):
    """Host-side graph preprocessing -> per-core device input maps."""
    bf16 = ml_dtypes.bfloat16
    row = np.asarray(edge_index[0], dtype=np.int64)
    col = np.asarray(edge_index[1], dtype=np.int64)
    w = np.asarray(edge_weight, dtype=np.float32)
    E = row.shape[0]

    deg = np.bincount(row, weights=w.astype(np.float64), minlength=N_NODES)
    deg = deg.astype(np.float32)
    dinv = np.where(deg > 0, np.where(deg > 0, deg, 1.0) ** -0.5, 0.0).astype(
        np.float32
    )
    lap2 = (-2.0 * dinv[row] * w * dinv[col]).astype(np.float32)

    # bucket edges by destination tile
    g = (row >> 7).astype(np.int64)
    order = np.argsort(g, kind="stable")
    rs, cs, ls, gs = row[order], col[order], lap2[order], g[order]
    counts = np.bincount(g, minlength=NTILES).astype(np.int64)
    KC = max(1, int(-(-counts.max() // 128)))
    NIDX = KC * 128
    offs = np.zeros(NTILES + 1, dtype=np.int64)
    np.cumsum(counts, out=offs[1:])
    slot = np.arange(E, dtype=np.int64) - offs[gs]

    idx_full = np.zeros((NTILES, NIDX), dtype=np.int16)          # pad: row 0
    dest_full = np.full((NTILES, NIDX), 300.0, dtype=np.float32)  # pad: no match
    val_full = np.zeros((NTILES, NIDX), dtype=np.float32)
    idx_full[gs, slot] = cs.astype(np.int16)
    dest_full[gs, slot] = (rs - (gs << 7)).astype(np.float32)
    val_full[gs, slot] = ls

    # gathered source rows: xrows[n] = [x[0,n,:], .., x[3,n,:]] bf16
    xrows = np.ascontiguousarray(
        np.transpose(np.asarray(x, np.float32), (1, 0, 2)).reshape(N_NODES, BF)
    ).astype(bf16)
    xn_pad = np.zeros((NPAD, BF), dtype=np.float32)
    xn_pad[:N_NODES] = xrows.astype(np.float32)

    W = np.asarray(weight, dtype=np.float32)
    Bm = W[1] + 2.0 * W[2] + W[3]
    A = (W[0] - W[2]) + 2.0 * SELF_LOOP_FILL * Bm
    biasv = np.asarray(bias, dtype=np.float32).reshape(128, 1)
    identity = np.eye(128, dtype=np.float32).astype(bf16)
    iota = np.tile(np.arange(128, dtype=np.float32), (128, 1)).astype(bf16)

    in_maps = []
    for c in range(NCORES):
        t0 = c * MT
        # idx layout: slot i of tile t -> partition i%16 (replicated x8), col i//16
        idx_core = (
            idx_full[t0 : t0 + MT]
            .reshape(MT, 8 * KC, 16)
            .transpose(2, 0, 1)
            .reshape(16, MT * 8 * KC)
        )
        idx_core = np.ascontiguousarray(np.tile(idx_core, (8, 1)))
        # dest/val layout: slot k*128+p of tile t -> [p, t, k]
        dest_core = np.ascontiguousarray(
            dest_full[t0 : t0 + MT].reshape(MT, KC, 128).transpose(2, 0, 1)
        ).astype(bf16)
        val_core = np.ascontiguousarray(
            val_full[t0 : t0 + MT].reshape(MT, KC, 128).transpose(2, 0, 1)
        ).astype(bf16)
        r0 = c * MROWS
        xtc = np.ascontiguousarray(
            xn_pad[r0 : r0 + MROWS].reshape(MROWS, B, F).transpose(2, 1, 0)
        ).astype(bf16)
        in_maps.append(
            {
                "xrows": xrows,
                "idxt": idx_core,
                "destt": dest_core,
                "valt": val_core,
                "xt": xtc,
                "wa": A.astype(bf16),
                "wb": Bm.astype(bf16),
                "biasv": biasv,
                "ident": identity,
                "iotab": iota,
            }
        )
    return KC, in_maps


def _ensure_ntff_hook():
    """Register the axon NTFF profiling hook if the image's antenv lacks it."""
    import sys
    import types

    try:
        from antenv.axon_hooks import get_axon_ntff_profile_hook  # noqa: F401

        return
    except ImportError:
        pass
    mod = types.ModuleType("antenv.axon_hooks")
    holder = {}
    mod.set_axon_ntff_profile_hook = lambda h: holder.__setitem__("h", h)
    mod.get_axon_ntff_profile_hook = lambda: holder.get("h")
    sys.modules["antenv.axon_hooks"] = mod
    import antenv

    antenv.axon_hooks = mod
    from trn_agent_boot.trn_boot import _ntff_profile_via_ctypes

    hook = _ntff_profile_via_ctypes("/opt/axon/libaxon_pjrt.so")
    if hook is not None:
        mod.set_axon_ntff_profile_hook(hook)


def kernel(x, edge_index, edge_weight, weight, bias):
    import os

    from concourse.bass_utils import run_bass_kernel_spmd

    x = np.asarray(x, dtype=np.float32)
    KC, in_maps = _prep_inputs(x, edge_index, edge_weight, weight, bias)
    nc = _get_nc(KC)
    trace = bool(int(os.environ.get("CHEB_TRACE", "0")))
    if trace:
        _ensure_ntff_hook()
    res = run_bass_kernel_spmd(nc, in_maps, list(range(NCORES)), trace=trace)
    _state["last_result"] = res
    out_T = np.concatenate([res.results[c]["out_t"] for c in range(NCORES)], axis=2)
    out = np.ascontiguousarray(out_T.transpose(0, 2, 1)[:, :N_NODES, :])
    return out
